# revision 8
# baseline (speedup 1.0000x reference)
"""Multi-head self-attention (B=4, T=2048, D=1024, H=16) on 8 Trainium2
NeuronCores, head-parallel (2 heads per core).

Per-core dataflow (bf16 matmuls, fp32 PSUM):
  xT[b] (host-pretransposed [D, T] bf16) -> SBUF
  qT/kT = w_{q,k}^T @ x^T        [128=2*dk, T]
  v     = x @ w_v                [T, 128], +ones col per head (l rides PV)
  S^T   = kT.T @ qT per (k-block, q-panel), 2 heads row-tiled
  causal: strictly-upper k-blocks skipped; on the diagonal superblock the
  scores/exp/PV are N-trimmed to the live q-range and only the [128,128]
  staircase subblock gets a 0/1 mask multiply
  P^T   = exp(S^T/8) on ACT (scalar engine does exp ONLY)
  PV    = v_aug.T @ P^T -> [65, W] PSUM; row 64 = softmax denominator l
  scale: recip_approx_fast(l) from PSUM -> gpsimd partition_broadcast ->
  DVE mul straight from PV PSUM into attnT (head1 shifted via DMA)
  proj  = attnT.T @ w_proj rows, emitted per panel -> fp16 partials -> HBM
Emission is interleaved: qkv(b+1) matmul chains are woven between
attention(b) iterations so the PE queue never drains behind exp.
Host: verifies causal mask, pre-transposes/casts x, sums 8 fp16 partials.
"""
import numpy as np
import ml_dtypes

B, T, D, H, DK = 4, 2048, 1024, 16, 64
NCORES = 8
CD = 128          # per-core head dims (2 heads x 64)
W = 512           # q panel width
NCH = D // 128    # contraction chunks for qkv
VS = 66           # v_aug per-head stride: 64 v cols + 1 ones + 1 pad

bf16 = ml_dtypes.bfloat16
_PROG_CACHE = {}
LAST_RESULT = None


def _install_ntff_hook():
    """Register antenv.axon_hooks (NTFF profiling) if the image lacks it."""
    import contextlib
    import ctypes
    import sys
    import types

    try:
        from antenv.axon_hooks import get_axon_ntff_profile_hook  # noqa: F401
        return
    except ImportError:
        pass

    lib = ctypes.CDLL("/opt/axon/libaxon_pjrt.so")
    if not hasattr(lib, "axon_start_nrt_profile"):
        return
    lib.axon_start_nrt_profile.argtypes = [ctypes.POINTER(ctypes.c_int64), ctypes.c_size_t]
    lib.axon_start_nrt_profile.restype = ctypes.c_int64
    lib.axon_stop_nrt_profile.argtypes = [ctypes.c_char_p]
    lib.axon_stop_nrt_profile.restype = ctypes.c_int64

    @contextlib.contextmanager
    def hook(output_dir, device_ids=None):
        import jax

        jax.devices()
        if device_ids:
            ids = (ctypes.c_int64 * len(device_ids))(*device_ids)
            rc = lib.axon_start_nrt_profile(ids, len(device_ids))
        else:
            rc = lib.axon_start_nrt_profile(None, 0)
        if rc != 0:
            raise RuntimeError(f"axon_start_nrt_profile rc={rc}")
        try:
            yield
        finally:
            n = lib.axon_stop_nrt_profile(str(output_dir).encode())
            print(f"profile: {n} file(s) written to {output_dir}", file=sys.stderr)

    mod = types.ModuleType("antenv.axon_hooks")
    mod.get_axon_ntff_profile_hook = lambda: hook
    mod.set_axon_ntff_profile_hook = lambda h: None
    sys.modules["antenv.axon_hooks"] = mod
    import antenv

    antenv.axon_hooks = mod


def build_program(Bv=B, Tv=T):
    import concourse.mybir as mybir
    import concourse.tile as tile
    from concourse import bacc, library_config

    dt = mybir.dt
    f32, b16, f16 = dt.float32, dt.bfloat16, dt.float16
    NPANEL = Tv // W
    NTOK = Tv // 128

    nc = bacc.Bacc()
    xt_d = nc.declare_dram_parameter("xt", [Bv, D, Tv], b16, isOutput=False)
    wq_d = nc.declare_dram_parameter("wq", [D, CD], b16, isOutput=False)
    wk_d = nc.declare_dram_parameter("wk", [D, CD], b16, isOutput=False)
    wv_d = nc.declare_dram_parameter("wv", [D, CD], b16, isOutput=False)
    wp_d = nc.declare_dram_parameter("wp", [CD, D], b16, isOutput=False)
    mk_d = nc.declare_dram_parameter("maskt", [128, 128], b16, isOutput=False)
    out_d = nc.declare_dram_parameter("out", [Bv, Tv, D], f16, isOutput=True)

    Exp = mybir.ActivationFunctionType.Exp

    with tile.TileContext(nc) as tc:
        with (
            tc.tile_pool(name="const", bufs=1) as constp,
            tc.tile_pool(name="xt", bufs=2) as xtp,
            tc.tile_pool(name="qk", bufs=2) as qkp,
            tc.tile_pool(name="vv", bufs=2) as vvp,
            tc.tile_pool(name="at", bufs=2) as atp,
            tc.tile_pool(name="pt", bufs=6) as ptp,
            tc.tile_pool(name="rc", bufs=4) as rcpp,
            tc.tile_pool(name="bc", bufs=4) as bcp,
            tc.tile_pool(name="stg", bufs=2) as stgp,
            tc.tile_pool(name="osb", bufs=3) as osbp,
            tc.tile_pool(name="mm", bufs=2, space="PSUM") as mmp,
            tc.tile_pool(name="qs", bufs=2, space="PSUM") as qsp,
            tc.tile_pool(name="pv", bufs=2, space="PSUM") as pvp,
        ):
            nc.gpsimd.load_library(library_config.proxy)

            # --- constants ---
            wq_sb = constp.tile([128, NCH * CD], b16, tag="wq")
            wk_sb = constp.tile([128, NCH * CD], b16, tag="wk")
            wv_sb = constp.tile([128, NCH * CD], b16, tag="wv")
            for w_d, w_sb in ((wq_d, wq_sb), (wk_d, wk_sb), (wv_d, wv_sb)):
                nc.scalar.dma_start(
                    w_sb[:].rearrange("p (c m) -> p c m", c=NCH),
                    w_d[:].rearrange("(c p) m -> p c m", p=128))
            wp_sb = constp.tile([128, D], b16, tag="wp")
            nc.scalar.dma_start(wp_sb[:], wp_d[:])
            # [128, 128] staircase: mask[k, q] = 1 if q >= k (within block)
            mask_sb = constp.tile([128, 128], b16, tag="mask")
            nc.scalar.dma_start(mask_sb[:], mk_d[:])

            state = {}

            def emit_qkv_gen(b):
                """Generator: qkv for batch b in panel-major weave pieces —
                xt quarter-loads, then the q/k/v chains of that panel, so
                attention on panel p can start as soon as group p lands."""
                xt_sb = xtp.tile([128, NCH * Tv], b16, tag="xt")
                qT = qkp.tile([128, Tv], b16, tag="qT")
                kT = qkp.tile([128, Tv], b16, tag="kT")
                v_sb = vvp.tile([128, NTOK * 2 * VS], b16, tag="v")
                vr = v_sb[:].rearrange("p (n h s) -> p n h s", h=2, s=VS)
                nc.vector.memset(vr[:, :, :, 64:65], 1.0)
                state[b] = {"qT": qT, "kT": kT, "vr": vr}
                for p in range(NPANEL):
                    for ch in range(NCH):
                        nc.sync.dma_start(
                            xt_sb[:, ch * Tv + p * W: ch * Tv + (p + 1) * W],
                            xt_d[b, ch * 128:(ch + 1) * 128, p * W:(p + 1) * W])
                    yield
                    for w_sb, dst in ((wq_sb, qT), (wk_sb, kT)):
                        ps = mmp.tile([128, W], f32, tag="mm")
                        for ch in range(NCH):
                            nc.tensor.matmul(
                                ps[:], w_sb[:, ch * CD:(ch + 1) * CD],
                                xt_sb[:, ch * Tv + p * W: ch * Tv + (p + 1) * W],
                                start=(ch == 0), stop=(ch == NCH - 1))
                        nc.vector.tensor_copy(dst[:, p * W:(p + 1) * W], ps[:])
                        yield
                    kb0 = 4 * p
                    ps = mmp.tile([128, 4 * CD], f32, tag="mm", name="vps")
                    for kb in range(kb0, kb0 + 4):
                        for ch in range(NCH):
                            nc.tensor.matmul(
                                ps[:, (kb - kb0) * CD:(kb - kb0 + 1) * CD],
                                xt_sb[:, ch * Tv + kb * 128: ch * Tv + kb * 128 + 128],
                                wv_sb[:, ch * CD:(ch + 1) * CD],
                                start=(ch == 0), stop=(ch == NCH - 1))
                        if kb == kb0 + 1:
                            yield
                    nc.vector.tensor_copy(
                        vr[:, kb0:kb0 + 4, :, 0:64],
                        ps[:].rearrange("p (n h s) -> p n h s", h=2, s=64))
                    yield

            def emit_batch(b, gen, reverse=False):
                """Attention + per-panel scale/proj for batch b, weaving qkv
                pieces from gen (batch b+1) between iterations. reverse=True
                runs panels largest-first so the final panel's serial tail is
                the shortest one (used for the last batch)."""
                st = state[b]
                qT, kT, vr = st["qT"], st["kT"], st["vr"]
                attnT = atp.tile([128, Tv], b16, tag="attnT")

                def weave():
                    if gen is not None:
                        next(gen, None)

                panels = range(NPANEL - 1, -1, -1) if reverse else range(NPANEL)
                for p in panels:
                    nkb = 4 * (p + 1)
                    pv_ps = [pvp.tile([65, W], f32, tag="pv", name=f"pv{h}")
                             for h in range(2)]
                    pts = {}

                    def emit_pv(kb, nkb=nkb, pv_ps=pv_ps, pts=pts):
                        pt, o = pts.pop(kb)
                        for h in range(2):
                            nc.tensor.matmul(
                                pv_ps[h][0:65, o:W], vr[:, kb, h, 0:65],
                                pt[:, h * W + o:(h + 1) * W],
                                start=(kb == 0), stop=(kb == nkb - 1),
                                skip_group_check=True)

                    for kb in range(nkb):
                        j = kb - 4 * p           # >= 0 on the diagonal superblock
                        o = max(j, 0) * 128      # live q-range starts here
                        qk = qsp.tile([128, 2 * W], f32, tag="qk")
                        for h in range(2):
                            nc.tensor.matmul(
                                qk[:, h * W + o:(h + 1) * W],
                                kT[64 * h:64 * (h + 1), kb * 128:(kb + 1) * 128],
                                qT[64 * h:64 * (h + 1), p * W + o:(p + 1) * W],
                                start=True, stop=True, tile_position=(64 * h, 0))
                        pt = ptp.tile([128, 2 * W], b16, tag="pt")
                        if o == 0:
                            nc.scalar.activation(pt[:], qk[:], Exp, scale=0.125)
                        else:
                            qv = qk[:].rearrange("p (h q) -> p h q", h=2)[:, :, o:W]
                            pv_ = pt[:].rearrange("p (h q) -> p h q", h=2)[:, :, o:W]
                            nc.scalar.activation(pv_, qv, Exp, scale=0.125)
                        if j >= 0:
                            for h in range(2):
                                nc.vector.tensor_mul(
                                    pt[:, h * W + o:h * W + o + 128],
                                    pt[:, h * W + o:h * W + o + 128],
                                    mask_sb[:])
                        pts[kb] = (pt, o)
                        if kb >= 2:
                            emit_pv(kb - 2)
                        weave()
                    emit_pv(nkb - 2)
                    emit_pv(nkb - 1)

                    # --- l -> recip -> broadcast -> scale into attnT ---
                    # both heads' chains interleaved to overlap latencies;
                    # l row: PSUM[64] -> SBUF[64] (aligned copy), DMA-shift to
                    # partition 0 (scalar queue: tiny, keeps sync free for xt)
                    lrows, l0s, rcps = [], [], []
                    for h in range(2):
                        lrow = rcpp.tile([65, W], f32, tag="lrow")
                        nc.vector.tensor_copy(lrow[64:65, :], pv_ps[h][64:65, :])
                        lrows.append(lrow)
                    for h in range(2):
                        l0 = rcpp.tile([1, W], f32, tag="l0")
                        nc.scalar.dma_start(l0[:], lrows[h][64:65, :])
                        l0s.append(l0)
                    for h in range(2):
                        rcp = rcpp.tile([1, W], f32, tag="rcp")
                        nc.vector.reciprocal_approx_fast(rcp[:], l0s[h][:])
                        rcps.append(rcp)
                    bcs = []
                    for h in range(2):
                        bc = bcp.tile([64, W], f32, tag="bc")
                        nc.gpsimd.partition_broadcast(bc[:], rcps[h][0:1, :], channels=64)
                        bcs.append(bc)
                    nc.vector.tensor_mul(
                        attnT[0:64, p * W:(p + 1) * W], pv_ps[0][0:64, :], bcs[0][:])
                    stg = stgp.tile([64, W], b16, tag="stg")
                    nc.vector.tensor_mul(stg[:], pv_ps[1][0:64, :], bcs[1][:])
                    nc.gpsimd.dma_start(attnT[64:128, p * W:(p + 1) * W], stg[:])

                    # --- proj for this panel's token blocks ---
                    for j in range(4 * p, 4 * p + 4):
                        osb = osbp.tile([128, D], f16, tag="osb")
                        for n in range(D // W):
                            ps = mmp.tile([128, W], f32, tag="mm", name="pj")
                            nc.tensor.matmul(
                                ps[:], attnT[:, j * 128:(j + 1) * 128],
                                wp_sb[:, n * W:(n + 1) * W], start=True, stop=True)
                            nc.vector.tensor_copy(osb[:, n * W:(n + 1) * W], ps[:])
                        eng = nc.gpsimd if j % 2 == 0 else nc.sync
                        eng.dma_start(out_d[b, j * 128:(j + 1) * 128, :], osb[:])
                        weave()
                del state[b]

            import itertools

            # Prefill: batch 0's panel-0 loads + q/k/v chains, then weave the
            # rest of batch 0's qkv together with batch 1's into attention(0).
            gen0 = emit_qkv_gen(0)
            for _ in range(4):
                next(gen0)
            pending = gen0
            for b in range(Bv):
                if b + 1 < Bv:
                    pending = itertools.chain(pending, emit_qkv_gen(b + 1))
                emit_batch(b, pending, reverse=(b == Bv - 1))
                for _ in pending:
                    pass
                pending = iter(())

    nc.compile()
    return nc


def prep_core_inputs(x, attn_mask, w_qkv, w_proj):
    """Host-side shard prep. Returns list of 8 in_maps."""
    Bv, Tv, Dv = x.shape
    xt = np.ascontiguousarray(x.transpose(0, 2, 1)).astype(bf16)
    kl = np.arange(128)
    ql = np.arange(128)
    maskt = (ql[None, :] >= kl[:, None]).astype(bf16)  # [k, q] staircase
    in_maps = []
    for c in range(NCORES):
        in_maps.append({
            "xt": xt,
            "wq": np.ascontiguousarray(w_qkv[:, CD * c:CD * (c + 1)]).astype(bf16),
            "wk": np.ascontiguousarray(w_qkv[:, Dv + CD * c:Dv + CD * (c + 1)]).astype(bf16),
            "wv": np.ascontiguousarray(w_qkv[:, 2 * Dv + CD * c:2 * Dv + CD * (c + 1)]).astype(bf16),
            "wp": np.ascontiguousarray(w_proj[CD * c:CD * (c + 1), :]).astype(bf16),
            "maskt": np.ascontiguousarray(maskt),
        })
    return in_maps


def check_causal(attn_mask):
    m = np.asarray(attn_mask)[0, 0]
    Tv = m.shape[0]
    tril = np.tril(np.ones((Tv, Tv), bool))
    return bool(np.all(m[tril] == 0.0)) and bool(np.all(m[~tril] <= np.float32(-1e30)))


def kernel(x, attn_mask, w_qkv, w_proj):
    import os

    from concourse.bass_utils import run_bass_kernel_spmd

    global LAST_RESULT
    x = np.asarray(x)
    attn_mask = np.asarray(attn_mask)
    w_qkv = np.asarray(w_qkv)
    w_proj = np.asarray(w_proj)
    if not check_causal(attn_mask):
        raise NotImplementedError("kernel compiled for causal attn_mask")

    key = (x.shape[0], x.shape[1])
    if key not in _PROG_CACHE:
        _PROG_CACHE[key] = build_program(Bv=x.shape[0], Tv=x.shape[1])
    nc = _PROG_CACHE[key]

    in_maps = prep_core_inputs(x, attn_mask, w_qkv, w_proj)
    kwargs = {}
    if os.environ.get("MHSA_TRACE"):
        _install_ntff_hook()
        kwargs = {"trace": True, "tmpdir": os.environ.get("MHSA_TRACE_DIR") or None}
    res = run_bass_kernel_spmd(nc, in_maps, list(range(NCORES)), **kwargs)
    LAST_RESULT = res
    out = np.zeros((x.shape[0], x.shape[1], D), np.float32)
    for c in range(NCORES):
        out += res.results[c]["out"].astype(np.float32)
    return out


# revision 10
# speedup vs baseline: 1.0331x; 1.0331x over previous
"""Multi-head self-attention (B=4, T=2048, D=1024, H=16) on 8 Trainium2
NeuronCores, head-parallel (2 heads per core).

Per-core dataflow (bf16 matmuls, fp32 PSUM):
  xT[b] (host-pretransposed [D, T] bf16) -> SBUF
  qT/kT = w_{q,k}^T @ x^T        [128=2*dk, T]
  v     = x @ w_v                [T, 128], +ones col per head (l rides PV)
  S^T   = kT.T @ qT per (k-block, q-panel), 2 heads row-tiled
  causal: strictly-upper k-blocks skipped; on the diagonal superblock the
  scores/exp/PV are N-trimmed to the live q-range and only the [128,128]
  staircase subblock gets a 0/1 mask multiply
  P^T   = exp(S^T/8) on ACT (scalar engine does exp ONLY)
  PV    = v_aug.T @ P^T -> [65, W] PSUM; row 64 = softmax denominator l
  scale: recip_approx_fast(l) from PSUM -> gpsimd partition_broadcast ->
  DVE mul straight from PV PSUM into attnT (head1 shifted via DMA)
  proj  = attnT.T @ w_proj rows, emitted per panel -> fp16 partials -> HBM
Emission is interleaved: qkv(b+1) matmul chains are woven between
attention(b) iterations so the PE queue never drains behind exp.
Host: verifies causal mask, pre-transposes/casts x, sums 8 fp16 partials.
"""
import numpy as np
import ml_dtypes

B, T, D, H, DK = 4, 2048, 1024, 16, 64
NCORES = 8
CD = 128          # per-core head dims (2 heads x 64)
W = 512           # q panel width
NCH = D // 128    # contraction chunks for qkv
VS = 66           # v_aug per-head stride: 64 v cols + 1 ones + 1 pad

bf16 = ml_dtypes.bfloat16
_PROG_CACHE = {}
LAST_RESULT = None


def _install_ntff_hook():
    """Register antenv.axon_hooks (NTFF profiling) if the image lacks it."""
    import contextlib
    import ctypes
    import sys
    import types

    try:
        from antenv.axon_hooks import get_axon_ntff_profile_hook  # noqa: F401
        return
    except ImportError:
        pass

    lib = ctypes.CDLL("/opt/axon/libaxon_pjrt.so")
    if not hasattr(lib, "axon_start_nrt_profile"):
        return
    lib.axon_start_nrt_profile.argtypes = [ctypes.POINTER(ctypes.c_int64), ctypes.c_size_t]
    lib.axon_start_nrt_profile.restype = ctypes.c_int64
    lib.axon_stop_nrt_profile.argtypes = [ctypes.c_char_p]
    lib.axon_stop_nrt_profile.restype = ctypes.c_int64

    @contextlib.contextmanager
    def hook(output_dir, device_ids=None):
        import jax

        jax.devices()
        if device_ids:
            ids = (ctypes.c_int64 * len(device_ids))(*device_ids)
            rc = lib.axon_start_nrt_profile(ids, len(device_ids))
        else:
            rc = lib.axon_start_nrt_profile(None, 0)
        if rc != 0:
            raise RuntimeError(f"axon_start_nrt_profile rc={rc}")
        try:
            yield
        finally:
            n = lib.axon_stop_nrt_profile(str(output_dir).encode())
            print(f"profile: {n} file(s) written to {output_dir}", file=sys.stderr)

    mod = types.ModuleType("antenv.axon_hooks")
    mod.get_axon_ntff_profile_hook = lambda: hook
    mod.set_axon_ntff_profile_hook = lambda h: None
    sys.modules["antenv.axon_hooks"] = mod
    import antenv

    antenv.axon_hooks = mod


def build_program(Bv=B, Tv=T):
    import concourse.mybir as mybir
    import concourse.tile as tile
    from concourse import bacc, library_config

    dt = mybir.dt
    f32, b16, f16 = dt.float32, dt.bfloat16, dt.float16
    NPANEL = Tv // W
    NTOK = Tv // 128

    nc = bacc.Bacc()
    xt_d = nc.declare_dram_parameter("xt", [Bv, D, Tv], b16, isOutput=False)
    wq_d = nc.declare_dram_parameter("wq", [D, CD], b16, isOutput=False)
    wk_d = nc.declare_dram_parameter("wk", [D, CD], b16, isOutput=False)
    wv_d = nc.declare_dram_parameter("wv", [D, CD], b16, isOutput=False)
    wp_d = nc.declare_dram_parameter("wp", [CD, D], b16, isOutput=False)
    mk_d = nc.declare_dram_parameter("maskt", [128, 128], b16, isOutput=False)
    out_d = nc.declare_dram_parameter("out", [Bv, Tv, D], f16, isOutput=True)

    Exp = mybir.ActivationFunctionType.Exp

    with tile.TileContext(nc) as tc:
        with (
            tc.tile_pool(name="const", bufs=1) as constp,
            tc.tile_pool(name="xt", bufs=2) as xtp,
            tc.tile_pool(name="qk", bufs=2) as qkp,
            tc.tile_pool(name="vv", bufs=2) as vvp,
            tc.tile_pool(name="at", bufs=2) as atp,
            tc.tile_pool(name="pt", bufs=6) as ptp,
            tc.tile_pool(name="rc", bufs=4) as rcpp,
            tc.tile_pool(name="bc", bufs=4) as bcp,
            tc.tile_pool(name="stg", bufs=2) as stgp,
            tc.tile_pool(name="osb", bufs=3) as osbp,
            tc.tile_pool(name="mm", bufs=2, space="PSUM") as mmp,
            tc.tile_pool(name="qs", bufs=2, space="PSUM") as qsp,
            tc.tile_pool(name="pv", bufs=2, space="PSUM") as pvp,
        ):
            nc.gpsimd.load_library(library_config.proxy)

            # --- constants ---
            wq_sb = constp.tile([128, NCH * CD], b16, tag="wq")
            wk_sb = constp.tile([128, NCH * CD], b16, tag="wk")
            wv_sb = constp.tile([128, NCH * CD], b16, tag="wv")
            for w_d, w_sb in ((wq_d, wq_sb), (wk_d, wk_sb), (wv_d, wv_sb)):
                nc.scalar.dma_start(
                    w_sb[:].rearrange("p (c m) -> p c m", c=NCH),
                    w_d[:].rearrange("(c p) m -> p c m", p=128))
            wp_sb = constp.tile([128, D], b16, tag="wp")
            nc.scalar.dma_start(wp_sb[:], wp_d[:])
            # [128, 128] staircase: mask[k, q] = 1 if q >= k (within block)
            mask_sb = constp.tile([128, 128], b16, tag="mask")
            nc.scalar.dma_start(mask_sb[:], mk_d[:])

            state = {}

            def emit_qkv_gen(b):
                """Generator: qkv for batch b in panel-major weave pieces —
                xt quarter-loads, then the q/k/v chains of that panel, so
                attention on panel p can start as soon as group p lands."""
                xt_sb = xtp.tile([128, NCH * Tv], b16, tag="xt")
                qT = qkp.tile([128, Tv], b16, tag="qT")
                kT = qkp.tile([128, Tv], b16, tag="kT")
                v_sb = vvp.tile([128, NTOK * 2 * VS], b16, tag="v")
                vr = v_sb[:].rearrange("p (n h s) -> p n h s", h=2, s=VS)
                nc.vector.memset(vr[:, :, :, 64:65], 1.0)
                state[b] = {"qT": qT, "kT": kT, "vr": vr}
                for p in range(NPANEL):
                    for ch in range(NCH):
                        nc.sync.dma_start(
                            xt_sb[:, ch * Tv + p * W: ch * Tv + (p + 1) * W],
                            xt_d[b, ch * 128:(ch + 1) * 128, p * W:(p + 1) * W])
                    yield
                    for w_sb, dst in ((wq_sb, qT), (wk_sb, kT)):
                        ps = mmp.tile([128, W], f32, tag="mm")
                        for ch in range(NCH):
                            nc.tensor.matmul(
                                ps[:], w_sb[:, ch * CD:(ch + 1) * CD],
                                xt_sb[:, ch * Tv + p * W: ch * Tv + (p + 1) * W],
                                start=(ch == 0), stop=(ch == NCH - 1))
                        nc.vector.tensor_copy(dst[:, p * W:(p + 1) * W], ps[:])
                        yield
                    kb0 = 4 * p
                    ps = mmp.tile([128, 4 * CD], f32, tag="mm", name="vps")
                    for kb in range(kb0, kb0 + 4):
                        for ch in range(NCH):
                            nc.tensor.matmul(
                                ps[:, (kb - kb0) * CD:(kb - kb0 + 1) * CD],
                                xt_sb[:, ch * Tv + kb * 128: ch * Tv + kb * 128 + 128],
                                wv_sb[:, ch * CD:(ch + 1) * CD],
                                start=(ch == 0), stop=(ch == NCH - 1))
                        if kb == kb0 + 1:
                            yield
                    nc.vector.tensor_copy(
                        vr[:, kb0:kb0 + 4, :, 0:64],
                        ps[:].rearrange("p (n h s) -> p n h s", h=2, s=64))
                    yield

            def emit_batch(b, gen, reverse=False):
                """Attention + per-panel scale/proj for batch b, weaving qkv
                pieces from gen (batch b+1) between iterations. reverse=True
                runs panels largest-first so the final panel's serial tail is
                the shortest one (used for the last batch)."""
                st = state[b]
                qT, kT, vr = st["qT"], st["kT"], st["vr"]
                attnT = atp.tile([128, Tv], b16, tag="attnT")

                def weave():
                    if gen is not None:
                        next(gen, None)

                panels = range(NPANEL - 1, -1, -1) if reverse else range(NPANEL)
                for p in panels:
                    nkb = 4 * (p + 1)
                    pv_ps = [pvp.tile([65, W], f32, tag="pv", name=f"pv{h}")
                             for h in range(2)]
                    pts = {}

                    def emit_pv(kb, nkb=nkb, pv_ps=pv_ps, pts=pts):
                        pt, o = pts.pop(kb)
                        for h in range(2):
                            nc.tensor.matmul(
                                pv_ps[h][0:65, o:W], vr[:, kb, h, 0:65],
                                pt[:, h * W + o:(h + 1) * W],
                                start=(kb == 0), stop=(kb == nkb - 1),
                                skip_group_check=True)

                    for kb in range(nkb):
                        j = kb - 4 * p           # >= 0 on the diagonal superblock
                        o = max(j, 0) * 128      # live q-range starts here
                        qk = qsp.tile([128, 2 * W], f32, tag="qk")
                        for h in range(2):
                            nc.tensor.matmul(
                                qk[:, h * W + o:(h + 1) * W],
                                kT[64 * h:64 * (h + 1), kb * 128:(kb + 1) * 128],
                                qT[64 * h:64 * (h + 1), p * W + o:(p + 1) * W],
                                start=True, stop=True, tile_position=(64 * h, 0))
                        pt = ptp.tile([128, 2 * W], b16, tag="pt")
                        if o == 0:
                            nc.scalar.activation(pt[:], qk[:], Exp, scale=0.125)
                        else:
                            qv = qk[:].rearrange("p (h q) -> p h q", h=2)[:, :, o:W]
                            pv_ = pt[:].rearrange("p (h q) -> p h q", h=2)[:, :, o:W]
                            nc.scalar.activation(pv_, qv, Exp, scale=0.125)
                        if j >= 0:
                            for h in range(2):
                                nc.vector.tensor_mul(
                                    pt[:, h * W + o:h * W + o + 128],
                                    pt[:, h * W + o:h * W + o + 128],
                                    mask_sb[:])
                        pts[kb] = (pt, o)
                        if kb >= 2:
                            emit_pv(kb - 2)
                        weave()
                    emit_pv(nkb - 2)
                    emit_pv(nkb - 1)

                    # --- l -> recip -> broadcast -> scale into attnT ---
                    # both heads' chains interleaved to overlap latencies;
                    # l row: PSUM[64] -> SBUF[64] (aligned copy), DMA-shift to
                    # partition 0 (scalar queue: tiny, keeps sync free for xt)
                    lrows, l0s, rcps = [], [], []
                    for h in range(2):
                        lrow = rcpp.tile([65, W], f32, tag="lrow")
                        nc.vector.tensor_copy(lrow[64:65, :], pv_ps[h][64:65, :])
                        lrows.append(lrow)
                    for h in range(2):
                        l0 = rcpp.tile([1, W], f32, tag="l0")
                        nc.sync.dma_start(l0[:], lrows[h][64:65, :])
                        l0s.append(l0)
                    for h in range(2):
                        rcp = rcpp.tile([1, W], f32, tag="rcp")
                        nc.vector.reciprocal_approx_fast(rcp[:], l0s[h][:])
                        rcps.append(rcp)
                    bcs = []
                    for h in range(2):
                        bc = bcp.tile([64, W], f32, tag="bc")
                        nc.gpsimd.partition_broadcast(bc[:], rcps[h][0:1, :], channels=64)
                        bcs.append(bc)
                    nc.vector.tensor_mul(
                        attnT[0:64, p * W:(p + 1) * W], pv_ps[0][0:64, :], bcs[0][:])
                    stg = stgp.tile([64, W], b16, tag="stg")
                    nc.vector.tensor_mul(stg[:], pv_ps[1][0:64, :], bcs[1][:])
                    nc.gpsimd.dma_start(attnT[64:128, p * W:(p + 1) * W], stg[:])

                    # --- proj for this panel's token blocks ---
                    for j in range(4 * p, 4 * p + 4):
                        osb = osbp.tile([128, D], f16, tag="osb")
                        for n in range(D // W):
                            ps = mmp.tile([128, W], f32, tag="mm", name="pj")
                            nc.tensor.matmul(
                                ps[:], attnT[:, j * 128:(j + 1) * 128],
                                wp_sb[:, n * W:(n + 1) * W], start=True, stop=True)
                            nc.vector.tensor_copy(osb[:, n * W:(n + 1) * W], ps[:])
                        nc.gpsimd.dma_start(out_d[b, j * 128:(j + 1) * 128, :], osb[:])
                        weave()
                del state[b]

            import itertools

            # Prefill: batch 0's panel-0 loads + q/k/v chains, then weave the
            # rest of batch 0's qkv together with batch 1's into attention(0).
            gen0 = emit_qkv_gen(0)
            for _ in range(4):
                next(gen0)
            pending = gen0
            for b in range(Bv):
                if b + 1 < Bv:
                    pending = itertools.chain(pending, emit_qkv_gen(b + 1))
                emit_batch(b, pending, reverse=(b == Bv - 1))
                for _ in pending:
                    pass
                pending = iter(())

    nc.compile()
    return nc


def prep_core_inputs(x, attn_mask, w_qkv, w_proj):
    """Host-side shard prep. Returns list of 8 in_maps."""
    Bv, Tv, Dv = x.shape
    xt = np.ascontiguousarray(x.transpose(0, 2, 1)).astype(bf16)
    kl = np.arange(128)
    ql = np.arange(128)
    maskt = (ql[None, :] >= kl[:, None]).astype(bf16)  # [k, q] staircase
    in_maps = []
    for c in range(NCORES):
        in_maps.append({
            "xt": xt,
            "wq": np.ascontiguousarray(w_qkv[:, CD * c:CD * (c + 1)]).astype(bf16),
            "wk": np.ascontiguousarray(w_qkv[:, Dv + CD * c:Dv + CD * (c + 1)]).astype(bf16),
            "wv": np.ascontiguousarray(w_qkv[:, 2 * Dv + CD * c:2 * Dv + CD * (c + 1)]).astype(bf16),
            "wp": np.ascontiguousarray(w_proj[CD * c:CD * (c + 1), :]).astype(bf16),
            "maskt": np.ascontiguousarray(maskt),
        })
    return in_maps


def check_causal(attn_mask):
    m = np.asarray(attn_mask)[0, 0]
    Tv = m.shape[0]
    tril = np.tril(np.ones((Tv, Tv), bool))
    return bool(np.all(m[tril] == 0.0)) and bool(np.all(m[~tril] <= np.float32(-1e30)))


def kernel(x, attn_mask, w_qkv, w_proj):
    import os

    from concourse.bass_utils import run_bass_kernel_spmd

    global LAST_RESULT
    x = np.asarray(x)
    attn_mask = np.asarray(attn_mask)
    w_qkv = np.asarray(w_qkv)
    w_proj = np.asarray(w_proj)
    if not check_causal(attn_mask):
        raise NotImplementedError("kernel compiled for causal attn_mask")

    key = (x.shape[0], x.shape[1])
    if key not in _PROG_CACHE:
        _PROG_CACHE[key] = build_program(Bv=x.shape[0], Tv=x.shape[1])
    nc = _PROG_CACHE[key]

    in_maps = prep_core_inputs(x, attn_mask, w_qkv, w_proj)
    kwargs = {}
    if os.environ.get("MHSA_TRACE"):
        _install_ntff_hook()
        kwargs = {"trace": True, "tmpdir": os.environ.get("MHSA_TRACE_DIR") or None}
    res = run_bass_kernel_spmd(nc, in_maps, list(range(NCORES)), **kwargs)
    LAST_RESULT = res
    out = np.zeros((x.shape[0], x.shape[1], D), np.float32)
    for c in range(NCORES):
        out += res.results[c]["out"].astype(np.float32)
    return out


# revision 13
# speedup vs baseline: 1.1047x; 1.0692x over previous
"""Multi-head self-attention (B=4, T=2048, D=1024, H=16) on 8 Trainium2
NeuronCores, head-parallel (2 heads per core).

Per-core dataflow (bf16 matmuls, fp32 PSUM):
  xT[b] (host-pretransposed [D, T] bf16) -> SBUF
  qT/kT = w_{q,k}^T @ x^T        [128=2*dk, T]
  v     = x @ w_v                [T, 128], +ones col per head (l rides PV)
  S^T   = kT.T @ qT per (k-block, q-panel), 2 heads row-tiled
  causal: strictly-upper k-blocks skipped; on the diagonal superblock the
  scores/exp/PV are N-trimmed to the live q-range and only the [128,128]
  staircase subblock gets a 0/1 mask multiply
  P^T   = exp(S^T/8) on ACT (scalar engine does exp ONLY)
  PV    = v_aug.T @ P^T -> [65, W] PSUM; row 64 = softmax denominator l
  scale: recip_approx_fast(l) from PSUM -> gpsimd partition_broadcast ->
  DVE mul straight from PV PSUM into attnT (head1 shifted via DMA)
  proj  = attnT.T @ w_proj rows, emitted per panel -> fp16 partials -> HBM
Emission is interleaved: qkv(b+1) matmul chains are woven between
attention(b) iterations so the PE queue never drains behind exp.
Host: verifies causal mask, pre-transposes/casts x, sums 8 fp16 partials.
"""
import numpy as np
import ml_dtypes

B, T, D, H, DK = 4, 2048, 1024, 16, 64
NCORES = 8
CD = 128          # per-core head dims (2 heads x 64)
W = 512           # q panel width
NCH = D // 128    # contraction chunks for qkv
VS = 66           # v_aug per-head stride: 64 v cols + 1 ones + 1 pad

bf16 = ml_dtypes.bfloat16
_PROG_CACHE = {}
LAST_RESULT = None


def _install_ntff_hook():
    """Register antenv.axon_hooks (NTFF profiling) if the image lacks it."""
    import contextlib
    import ctypes
    import sys
    import types

    try:
        from antenv.axon_hooks import get_axon_ntff_profile_hook  # noqa: F401
        return
    except ImportError:
        pass

    lib = ctypes.CDLL("/opt/axon/libaxon_pjrt.so")
    if not hasattr(lib, "axon_start_nrt_profile"):
        return
    lib.axon_start_nrt_profile.argtypes = [ctypes.POINTER(ctypes.c_int64), ctypes.c_size_t]
    lib.axon_start_nrt_profile.restype = ctypes.c_int64
    lib.axon_stop_nrt_profile.argtypes = [ctypes.c_char_p]
    lib.axon_stop_nrt_profile.restype = ctypes.c_int64

    @contextlib.contextmanager
    def hook(output_dir, device_ids=None):
        import jax

        jax.devices()
        if device_ids:
            ids = (ctypes.c_int64 * len(device_ids))(*device_ids)
            rc = lib.axon_start_nrt_profile(ids, len(device_ids))
        else:
            rc = lib.axon_start_nrt_profile(None, 0)
        if rc != 0:
            raise RuntimeError(f"axon_start_nrt_profile rc={rc}")
        try:
            yield
        finally:
            n = lib.axon_stop_nrt_profile(str(output_dir).encode())
            print(f"profile: {n} file(s) written to {output_dir}", file=sys.stderr)

    mod = types.ModuleType("antenv.axon_hooks")
    mod.get_axon_ntff_profile_hook = lambda: hook
    mod.set_axon_ntff_profile_hook = lambda h: None
    sys.modules["antenv.axon_hooks"] = mod
    import antenv

    antenv.axon_hooks = mod


def build_program(Bv=B, Tv=T):
    import concourse.mybir as mybir
    import concourse.tile as tile
    from concourse import bacc, library_config

    dt = mybir.dt
    f32, b16, f16 = dt.float32, dt.bfloat16, dt.float16
    NPANEL = Tv // W
    NTOK = Tv // 128

    nc = bacc.Bacc()
    xt_d = nc.declare_dram_parameter("xt", [Bv, D, Tv], b16, isOutput=False)
    wq_d = nc.declare_dram_parameter("wq", [D, CD], b16, isOutput=False)
    wk_d = nc.declare_dram_parameter("wk", [D, CD], b16, isOutput=False)
    wv_d = nc.declare_dram_parameter("wv", [D, CD], b16, isOutput=False)
    wp_d = nc.declare_dram_parameter("wp", [CD, D], b16, isOutput=False)
    mk_d = nc.declare_dram_parameter("maskt", [128, 128], b16, isOutput=False)
    out_d = nc.declare_dram_parameter("out", [Bv, Tv, D], f16, isOutput=True)

    Exp = mybir.ActivationFunctionType.Exp

    with tile.TileContext(nc) as tc:
        with (
            tc.tile_pool(name="const", bufs=1) as constp,
            tc.tile_pool(name="xt", bufs=2) as xtp,
            tc.tile_pool(name="qk", bufs=2) as qkp,
            tc.tile_pool(name="vv", bufs=2) as vvp,
            tc.tile_pool(name="at", bufs=2) as atp,
            tc.tile_pool(name="pt", bufs=6) as ptp,
            tc.tile_pool(name="rc", bufs=4) as rcpp,
            tc.tile_pool(name="bc", bufs=4) as bcp,
            tc.tile_pool(name="stg", bufs=2) as stgp,
            tc.tile_pool(name="osb", bufs=3) as osbp,
            tc.tile_pool(name="mm", bufs=2, space="PSUM") as mmp,
            tc.tile_pool(name="qs", bufs=2, space="PSUM") as qsp,
            tc.tile_pool(name="pv", bufs=2, space="PSUM") as pvp,
        ):
            nc.gpsimd.load_library(library_config.proxy)

            # --- constants ---
            wq_sb = constp.tile([128, NCH * CD], b16, tag="wq")
            wk_sb = constp.tile([128, NCH * CD], b16, tag="wk")
            wv_sb = constp.tile([128, NCH * CD], b16, tag="wv")
            for w_d, w_sb in ((wq_d, wq_sb), (wk_d, wk_sb), (wv_d, wv_sb)):
                nc.scalar.dma_start(
                    w_sb[:].rearrange("p (c m) -> p c m", c=NCH),
                    w_d[:].rearrange("(c p) m -> p c m", p=128))
            wp_sb = constp.tile([128, D], b16, tag="wp")
            nc.scalar.dma_start(wp_sb[:], wp_d[:])
            # [128, 128] staircase: mask[k, q] = 1 if q >= k (within block)
            mask_sb = constp.tile([128, 128], b16, tag="mask")
            nc.scalar.dma_start(mask_sb[:], mk_d[:])

            state = {}

            def emit_qkv_gen(b):
                """Generator: qkv for batch b in panel-major weave pieces —
                xt quarter-loads, then the q/k/v chains of that panel, so
                attention on panel p can start as soon as group p lands."""
                xt_sb = xtp.tile([128, NCH * Tv], b16, tag="xt")
                qT = qkp.tile([128, Tv], b16, tag="qT")
                kT = qkp.tile([128, Tv], b16, tag="kT")
                v_sb = vvp.tile([128, NTOK * 2 * VS], b16, tag="v")
                vr = v_sb[:].rearrange("p (n h s) -> p n h s", h=2, s=VS)
                nc.vector.memset(vr[:, :, :, 64:65], 1.0)
                state[b] = {"qT": qT, "kT": kT, "vr": vr}
                for p in range(NPANEL):
                    for ch in range(NCH):
                        nc.sync.dma_start(
                            xt_sb[:, ch * Tv + p * W: ch * Tv + (p + 1) * W],
                            xt_d[b, ch * 128:(ch + 1) * 128, p * W:(p + 1) * W])
                    yield
                    for w_sb, dst in ((wq_sb, qT), (wk_sb, kT)):
                        ps = mmp.tile([128, W], f32, tag="mm")
                        for ch in range(NCH):
                            nc.tensor.matmul(
                                ps[:], w_sb[:, ch * CD:(ch + 1) * CD],
                                xt_sb[:, ch * Tv + p * W: ch * Tv + (p + 1) * W],
                                start=(ch == 0), stop=(ch == NCH - 1))
                        nc.vector.tensor_copy(dst[:, p * W:(p + 1) * W], ps[:])
                        yield
                    kb0 = 4 * p
                    ps = mmp.tile([128, 4 * CD], f32, tag="mm", name="vps")
                    for kb in range(kb0, kb0 + 4):
                        for ch in range(NCH):
                            nc.tensor.matmul(
                                ps[:, (kb - kb0) * CD:(kb - kb0 + 1) * CD],
                                xt_sb[:, ch * Tv + kb * 128: ch * Tv + kb * 128 + 128],
                                wv_sb[:, ch * CD:(ch + 1) * CD],
                                start=(ch == 0), stop=(ch == NCH - 1))
                        if kb == kb0 + 1:
                            yield
                    nc.vector.tensor_copy(
                        vr[:, kb0:kb0 + 4, :, 0:64],
                        ps[:].rearrange("p (n h s) -> p n h s", h=2, s=64))
                    yield

            def emit_proj(b, attnT, j, last_panel=False):
                """One token-block of the output projection. For the very
                last panel the casts go to the idle scalar engine and the
                stores are split per half across both DMA queues to shorten
                the drain."""
                osb = osbp.tile([128, D], f16, tag="osb")
                for n in range(D // W):
                    ps = mmp.tile([128, W], f32, tag="mm", name="pj")
                    nc.tensor.matmul(
                        ps[:], attnT[:, j * 128:(j + 1) * 128],
                        wp_sb[:, n * W:(n + 1) * W], start=True, stop=True)
                    if last_panel:
                        nc.scalar.copy(osb[:, n * W:(n + 1) * W], ps[:])
                        eng = nc.gpsimd if n == 0 else nc.sync
                        eng.dma_start(
                            out_d[b, j * 128:(j + 1) * 128, n * W:(n + 1) * W],
                            osb[:, n * W:(n + 1) * W])
                    else:
                        nc.vector.tensor_copy(osb[:, n * W:(n + 1) * W], ps[:])
                if not last_panel:
                    nc.gpsimd.dma_start(out_d[b, j * 128:(j + 1) * 128, :], osb[:])

            def emit_batch(b, gen, reverse=False, defer_proj=False, stride=1):
                """Attention + per-panel scale for batch b, weaving pieces
                from gen between iterations (every `stride` iterations).
                reverse=True runs panels largest-first so the final panel's
                serial tail is the shortest one (used for the last batch).
                defer_proj=True returns proj closures for the next batch's
                weave instead of emitting them here."""
                st = state[b]
                qT, kT, vr = st["qT"], st["kT"], st["vr"]
                attnT = atp.tile([128, Tv], b16, tag="attnT")
                proj_jobs = []
                nweave = [0]

                def weave():
                    nweave[0] += 1
                    if gen is not None and nweave[0] % stride == 0:
                        next(gen, None)

                panels = range(NPANEL - 1, -1, -1) if reverse else range(NPANEL)
                for p in panels:
                    nkb = 4 * (p + 1)
                    pv_ps = [pvp.tile([65, W], f32, tag="pv", name=f"pv{h}")
                             for h in range(2)]
                    pts = {}

                    def emit_pv(kb, nkb=nkb, pv_ps=pv_ps, pts=pts):
                        pt, o = pts.pop(kb)
                        for h in range(2):
                            nc.tensor.matmul(
                                pv_ps[h][0:65, o:W], vr[:, kb, h, 0:65],
                                pt[:, h * W + o:(h + 1) * W],
                                start=(kb == 0), stop=(kb == nkb - 1),
                                skip_group_check=True)

                    for kb in range(nkb):
                        j = kb - 4 * p           # >= 0 on the diagonal superblock
                        o = max(j, 0) * 128      # live q-range starts here
                        qk = qsp.tile([128, 2 * W], f32, tag="qk")
                        for h in range(2):
                            nc.tensor.matmul(
                                qk[:, h * W + o:(h + 1) * W],
                                kT[64 * h:64 * (h + 1), kb * 128:(kb + 1) * 128],
                                qT[64 * h:64 * (h + 1), p * W + o:(p + 1) * W],
                                start=True, stop=True, tile_position=(64 * h, 0))
                        pt = ptp.tile([128, 2 * W], b16, tag="pt")
                        if o == 0:
                            nc.scalar.activation(pt[:], qk[:], Exp, scale=0.125)
                        else:
                            qv = qk[:].rearrange("p (h q) -> p h q", h=2)[:, :, o:W]
                            pv_ = pt[:].rearrange("p (h q) -> p h q", h=2)[:, :, o:W]
                            nc.scalar.activation(pv_, qv, Exp, scale=0.125)
                        if j >= 0:
                            for h in range(2):
                                nc.vector.tensor_mul(
                                    pt[:, h * W + o:h * W + o + 128],
                                    pt[:, h * W + o:h * W + o + 128],
                                    mask_sb[:])
                        pts[kb] = (pt, o)
                        if kb >= 2:
                            emit_pv(kb - 2)
                        weave()
                    emit_pv(nkb - 2)
                    emit_pv(nkb - 1)

                    # --- l -> recip -> broadcast -> scale into attnT ---
                    # both heads' chains interleaved to overlap latencies;
                    # l row: PSUM[64] -> SBUF[64] (aligned copy), DMA-shift to
                    # partition 0 (scalar queue: tiny, keeps sync free for xt)
                    lrows, l0s, rcps = [], [], []
                    for h in range(2):
                        lrow = rcpp.tile([65, W], f32, tag="lrow")
                        nc.vector.tensor_copy(lrow[64:65, :], pv_ps[h][64:65, :])
                        lrows.append(lrow)
                    for h in range(2):
                        l0 = rcpp.tile([1, W], f32, tag="l0")
                        nc.sync.dma_start(l0[:], lrows[h][64:65, :])
                        l0s.append(l0)
                    for h in range(2):
                        rcp = rcpp.tile([1, W], f32, tag="rcp")
                        nc.vector.reciprocal_approx_fast(rcp[:], l0s[h][:])
                        rcps.append(rcp)
                    bcs = []
                    for h in range(2):
                        bc = bcp.tile([64, W], f32, tag="bc")
                        nc.gpsimd.partition_broadcast(bc[:], rcps[h][0:1, :], channels=64)
                        bcs.append(bc)
                    nc.vector.tensor_mul(
                        attnT[0:64, p * W:(p + 1) * W], pv_ps[0][0:64, :], bcs[0][:])
                    stg = stgp.tile([64, W], b16, tag="stg")
                    nc.vector.tensor_mul(stg[:], pv_ps[1][0:64, :], bcs[1][:])
                    nc.gpsimd.dma_start(attnT[64:128, p * W:(p + 1) * W], stg[:])

                    # --- proj for this panel's token blocks ---
                    last_panel = p == (0 if reverse else NPANEL - 1)
                    for j in range(4 * p, 4 * p + 4):
                        if defer_proj:
                            proj_jobs.append(
                                lambda b=b, attnT=attnT, j=j: emit_proj(b, attnT, j))
                        else:
                            emit_proj(b, attnT, j, last_panel=last_panel)
                            weave()
                del state[b]
                return proj_jobs

            import itertools

            def iter_jobs(jobs):
                for f in jobs:
                    f()
                    yield

            # Prefill: batch 0's panel-0 loads + q/k/v chains, then weave the
            # rest of batch 0's qkv together with batch 1's into attention(0).
            # Each batch's proj is deferred into the NEXT batch's weave so the
            # last batch's exp-paced attention has PE work to fill with.
            gen0 = emit_qkv_gen(0)
            for _ in range(5):
                next(gen0)
            pending = gen0
            carry_proj = []
            for b in range(Bv):
                last = b + 1 >= Bv
                if not last:
                    pending = itertools.chain(pending, emit_qkv_gen(b + 1))
                pending = itertools.chain(pending, iter_jobs(carry_proj))
                carry_proj = emit_batch(
                    b, pending, reverse=last, defer_proj=not last,
                    stride=2 if last else 1)
                for _ in pending:
                    pass
                pending = iter(())

    nc.compile()
    return nc


def prep_core_inputs(x, attn_mask, w_qkv, w_proj):
    """Host-side shard prep. Returns list of 8 in_maps."""
    Bv, Tv, Dv = x.shape
    xt = np.ascontiguousarray(x.transpose(0, 2, 1)).astype(bf16)
    kl = np.arange(128)
    ql = np.arange(128)
    maskt = (ql[None, :] >= kl[:, None]).astype(bf16)  # [k, q] staircase
    in_maps = []
    for c in range(NCORES):
        in_maps.append({
            "xt": xt,
            "wq": np.ascontiguousarray(w_qkv[:, CD * c:CD * (c + 1)]).astype(bf16),
            "wk": np.ascontiguousarray(w_qkv[:, Dv + CD * c:Dv + CD * (c + 1)]).astype(bf16),
            "wv": np.ascontiguousarray(w_qkv[:, 2 * Dv + CD * c:2 * Dv + CD * (c + 1)]).astype(bf16),
            "wp": np.ascontiguousarray(w_proj[CD * c:CD * (c + 1), :]).astype(bf16),
            "maskt": np.ascontiguousarray(maskt),
        })
    return in_maps


def check_causal(attn_mask):
    m = np.asarray(attn_mask)[0, 0]
    Tv = m.shape[0]
    tril = np.tril(np.ones((Tv, Tv), bool))
    return bool(np.all(m[tril] == 0.0)) and bool(np.all(m[~tril] <= np.float32(-1e30)))


def kernel(x, attn_mask, w_qkv, w_proj):
    import os

    from concourse.bass_utils import run_bass_kernel_spmd

    global LAST_RESULT
    x = np.asarray(x)
    attn_mask = np.asarray(attn_mask)
    w_qkv = np.asarray(w_qkv)
    w_proj = np.asarray(w_proj)
    if not check_causal(attn_mask):
        raise NotImplementedError("kernel compiled for causal attn_mask")

    key = (x.shape[0], x.shape[1])
    if key not in _PROG_CACHE:
        _PROG_CACHE[key] = build_program(Bv=x.shape[0], Tv=x.shape[1])
    nc = _PROG_CACHE[key]

    in_maps = prep_core_inputs(x, attn_mask, w_qkv, w_proj)
    kwargs = {}
    if os.environ.get("MHSA_TRACE"):
        _install_ntff_hook()
        kwargs = {"trace": True, "tmpdir": os.environ.get("MHSA_TRACE_DIR") or None}
    res = run_bass_kernel_spmd(nc, in_maps, list(range(NCORES)), **kwargs)
    LAST_RESULT = res
    out = np.zeros((x.shape[0], x.shape[1], D), np.float32)
    for c in range(NCORES):
        out += res.results[c]["out"].astype(np.float32)
    return out
